# revision 29
# baseline (speedup 1.0000x reference)
"""Trainium2 Bass kernel for the DCT-CNN expert core (raw-bass scheduled).

Reference computation (per 512x512 single-channel image):
  1. split into 4096 non-overlapping 8x8 patches
  2. 2D DCT per patch:  c = D @ p @ D^T
  3. conv3x3(1->16, SAME) + bias + relu on each 8x8 patch image
  4. conv3x3(16->32, SAME) + bias
  5. mean over spatial (8x8), then mean over patches  -> [B, 32]

Algebraic restructuring (validated to fp32 roundoff):
  - DCT + conv1 fold into a single [1024, 64] matrix W = M1 @ (D (x) D)
    with bias b1 broadcast per channel: h1 = relu(W @ p + b1h)
  - conv2 + spatial mean + patch mean fold into a [1024, 32] matrix applied
    to the per-image SUM of h1:  out[b] = (sum_patches h1)^T @ M2e + b2

Device schedule per core (2 images = 8192 patches, pure data parallel),
hand-synchronized with 11 counting semaphores instead of the Tile
scheduler (the Tile version allocates ~250 sems whose one-instruction-
per-sem teardown costs ~9us of tail):
  - 32 pipelined iterations; iteration it = (chunk k = it//4, group g =
    it%4) computes PSUM parity slot (it%2): 4 bf16 matmuls [K=64 -> 128ch
    x 512patch], image 0 on PE rows 0:64 -> banks {0,1|4,5}, image 1 on
    rows 64:128 -> banks {2,3|6,7}, interleaved so both PE sub-array
    halves stream concurrently.
  - drains: ScalarE activation(Relu+bias+accum) on image 0, VectorE
    scalar_tensor_tensor on image 1, in parallel; WAR back to the PE via
    counting sems (psum slot reused every 2 iterations).
  - input DMAs split over both HWDGE queues in per-iteration consumption
    order (sync: wts-k0, p0, aux, p2, wts-rest, p4, p6 / scalar: p1, p3,
    p5, p7, m2), one completion semaphore per DMA.  A warm-up activation
    in front of scalar's DMA triggers hoists the one-time Relu ACT-table
    load (~2.7us) into the DMA window.
  - tail: tensor_reduce of the 64 accumulators + 8 accumulated
    [128,2]x[128,32] matmuls + bias add + DMA out.
"""
import numpy as np

import concourse.bass as bass
import concourse.bacc as bacc
from concourse import mybir
from concourse.bass_utils import run_bass_kernel_spmd

N_CORES = 8
F32 = mybir.dt.float32
BF16 = mybir.dt.bfloat16

import ml_dtypes
NP_BF16 = np.dtype(ml_dtypes.bfloat16)

# ---------------------------------------------------------------- host math

def _dct_matrix(n=8):
    m = np.zeros((n, n), dtype=np.float64)
    for k in range(n):
        for t in range(n):
            if k == 0:
                m[k, t] = 1.0 / np.sqrt(n)
            else:
                m[k, t] = np.sqrt(2.0 / n) * np.cos(np.pi * k * (2 * t + 1) / (2.0 * n))
    return m


def _conv3x3_matrix(w):
    """Dense linear operator of a SAME 3x3 cross-correlation on 8x8 images."""
    O, I = w.shape[0], w.shape[1]
    M = np.zeros((O, 8, 8, I, 8, 8))
    for dy in range(3):
        for dx in range(3):
            ylo, yhi = max(0, 1 - dy), min(8, 9 - dy)
            xlo, xhi = max(0, 1 - dx), min(8, 9 - dx)
            for y in range(ylo, yhi):
                for x in range(xlo, xhi):
                    M[:, y, x, :, y + dy - 1, x + dx - 1] += w[:, :, dy, dx]
    return M.reshape(O * 64, I * 64)


def _build_weights(w1, b1, w2, b2):
    """Returns (Wt [64,1024], b1c [128,8], M2c [128,256], b2t [128,32]) f32."""
    D = _dct_matrix()
    KRON = np.kron(D, D)
    M1 = _conv3x3_matrix(w1.astype(np.float64))            # [1024, 64]
    M1K = M1 @ KRON                                        # [1024, 64]
    b1h = np.repeat(b1.astype(np.float64), 64)             # [1024]
    M2 = _conv3x3_matrix(w2.astype(np.float64))            # [2048, 1024]
    A2 = M2.reshape(32, 64, 1024).sum(axis=1)              # [32, 1024]
    M2e = A2.T / (64.0 * 4096.0)                           # [1024, 32]

    Wt = np.ascontiguousarray(M1K.T, dtype=np.float32)     # [64, 1024]
    b1c = np.ascontiguousarray(
        b1h.reshape(8, 128).T, dtype=np.float32)           # [128, 8]
    M2c = np.ascontiguousarray(
        M2e.reshape(8, 128, 32).transpose(1, 0, 2).reshape(128, 256)
    ).astype(NP_BF16)                                      # [128, 8*32] bf16
    b2t = np.ascontiguousarray(
        np.tile(b2.astype(np.float32), (128, 1)))          # [128, 32]
    return Wt, b1c, M2c, b2t


# ------------------------------------------------------------- device kernel

# aux layout (f32 columns): [0:8) b1 chunks | [8:40) b2 broadcast
# (M2e chunks ride a separate bf16 tensor m2 [128, 256])
AUXB1 = 0
AUXB2 = 8
AUXTOT = 40


def _build_nc():
    nc = bacc.Bacc("TRN2", target_bir_lowering=False, debug=False,
                   num_devices=N_CORES)
    p_d = nc.declare_dram_parameter("p", [128, 4096], BF16, isOutput=False)
    wts_d = nc.declare_dram_parameter("wts", [128, 1024], BF16, isOutput=False)
    aux_d = nc.declare_dram_parameter("aux", [128, AUXTOT], F32, isOutput=False)
    m2_d = nc.declare_dram_parameter("m2", [128, 256], BF16, isOutput=False)
    out_d = nc.declare_dram_parameter("out", [2, 32], F32, isOutput=True)

    wts_t = nc.alloc_sbuf_tensor("wts_t", [128, 1024], BF16).ap()
    p_t = nc.alloc_sbuf_tensor("p_t", [128, 4096], BF16).ap()
    aux_t = nc.alloc_sbuf_tensor("aux_t", [128, AUXTOT], F32).ap()
    m2_t = nc.alloc_sbuf_tensor("m2_t", [128, 256], BF16).ap()
    acc_t = nc.alloc_sbuf_tensor("acc_t", [128, 64], F32).ap()
    s_t = nc.alloc_sbuf_tensor("s_t", [128, 16], BF16).ap()
    out_sb = nc.alloc_sbuf_tensor("out_sb", [2, 32], F32).ap()
    warm_t = nc.alloc_sbuf_tensor("warm_t", [1, 1], F32).ap()

    PS = nc.alloc_psum_tensor("PS", [128, 4096], F32).ap()  # all 8 banks

    # one semaphore per DMA: transfers from one issuing queue can fan out
    # over several hardware DGE queues, so a shared counting sem can hit a
    # threshold while an individual transfer is still in flight.
    s_wk = [nc.alloc_semaphore(f"s_wk{j}") for j in range(4)]  # wts k-pair DMAs
    s_m2 = nc.alloc_semaphore("s_m2")    # m2 DMA done (==16)
    s_aux = nc.alloc_semaphore("s_aux")  # aux DMA done (==16)
    s_p = [nc.alloc_semaphore(f"s_p{q}") for q in range(8)]
    s_pea = nc.alloc_semaphore("s_pea")  # image-0 matmul pairs done
    s_peb = nc.alloc_semaphore("s_peb")  # image-1 matmul pairs done
    s_act = nc.alloc_semaphore("s_act")  # ScalarE drains done
    s_dve = nc.alloc_semaphore("s_dve")  # VectorE drains done
    s_acc = nc.alloc_semaphore("s_acc")  # ScalarE accum reads retired
    s_red = nc.alloc_semaphore("s_red")  # accumulator reduce done
    s_fin = nc.alloc_semaphore("s_fin")  # final matmuls done
    s_tt = nc.alloc_semaphore("s_tt")    # bias add done
    s_out = nc.alloc_semaphore("s_out")  # output DMA done
    all_sems = s_wk + [s_m2, s_aux] + s_p + [s_pea, s_peb, s_act,
                s_dve, s_acc, s_red, s_fin, s_tt, s_out]

    zero_ap = nc.const_aps.aps[(F32, 0.0)]  # [128, 1] zeros (preamble memset)

    # Semaphores are NOT zeroed at NEFF load: clear ours before any use
    # (the first execution otherwise reads garbage sem state and races).
    sem_nums = sorted(s.num for s in all_sems)
    assert sem_nums == list(range(sem_nums[0], sem_nums[0] + len(sem_nums)))
    sem_rng = range(sem_nums[0], sem_nums[-1] + 1)
    nc.gpsimd.sem_clear(sem_rng)
    nc.all_engine_barrier()

    # ---- input DMAs, consumption order, split across both HWDGE queues.
    # scalar queue leads with the warm-up activation: its auto-inserted
    # ACT-table load (~1.3us + ~1.4us drain) runs while DMAs stream.
    nc.scalar.activation(warm_t, zero_ap[0:1, 0:1],
                         mybir.ActivationFunctionType.Relu,
                         bias=0.0, scale=1.0)
    # aux rides the sync HWDGE queue: SWDGE (gpsimd) completion sems fire
    # at descriptor submission, which races the first bias read.
    # Queue plans tuned so each iteration's chunk pair lands just before
    # its matmuls need it (the scalar queue starts ~1.3us late behind the
    # ACT-table load):  sync: wts0, p0, aux, p2, wts-rest, p4, p6
    #                   scalar: warm, p1, p3, p5, p7, m2
    def _dma_p(eng, q):
        eng.dma_start(
            out=p_t[:, 512 * q:512 * q + 512],
            in_=p_d[:, 512 * q:512 * q + 512]).then_inc(s_p[q], 16)
    # DMA completion sems land ~3.4us after transfer end, in queue order,
    # and each transfer occupies ~650ns of queue regardless of size — so
    # the first iterations' gates (wts k0-k1 pair, p0, p1, aux, wts k2-k3)
    # take the earliest queue slots.  k-major iteration order means p2..p7
    # are not needed until iteration 8 (~10us later).
    def _dma_wk(eng, j):
        eng.dma_start(out=wts_t[:, 256 * j:256 * j + 256],
                      in_=wts_d[:, 256 * j:256 * j + 256]).then_inc(s_wk[j], 16)
    _dma_p(nc.sync, 0)
    nc.sync.dma_start(out=aux_t, in_=aux_d[:, :]).then_inc(s_aux, 16)
    _dma_wk(nc.sync, 1)
    _dma_p(nc.sync, 2)
    _dma_p(nc.sync, 4)
    _dma_p(nc.sync, 6)
    _dma_wk(nc.scalar, 0)
    _dma_p(nc.scalar, 1)
    _dma_wk(nc.scalar, 2)
    _dma_wk(nc.scalar, 3)
    _dma_p(nc.scalar, 3)
    _dma_p(nc.scalar, 5)
    _dma_p(nc.scalar, 7)
    nc.scalar.dma_start(out=m2_t, in_=m2_d[:, :]).then_inc(s_m2, 16)

    # ---- main loop: 32 iterations, PSUM parity slots of 4 banks each:
    # parity 0 -> image0 cols [0:1024) banks 0-1, image1 [1024:2048) 2-3
    # parity 1 -> image0 [2048:3072) banks 4-5, image1 [3072:4096) 6-7
    # DMA gates: iteration it<4 introduces chunks t=2it (scalar q) and
    # t=2it+1 (sync q); thresholds in units of 16 per completed DMA.
    for it in range(32):
        g, k = it // 8, it % 8
        par = it % 2
        A0 = 2048 * par
        B0 = 2048 * par + 1024
        t0, t1 = 2 * g, 2 * g + 1
        b1_ap = aux_t[:, AUXB1 + k:AUXB1 + k + 1]
        lhsA = wts_t[0:64, 128 * k:128 * k + 128]
        lhsB = wts_t[64:128, 128 * k:128 * k + 128]
        rhsA0 = p_t[0:64, 512 * t0:512 * t0 + 512]
        rhsB0 = p_t[64:128, 512 * t0:512 * t0 + 512]
        rhsA1 = p_t[0:64, 512 * t1:512 * t1 + 512]
        rhsB1 = p_t[64:128, 512 * t1:512 * t1 + 512]

        # PE order A0,A1,B0,B1: adjacent different-row-group pairs still
        # stream through both PE halves concurrently, and the B-bank
        # overwrite starts ~600ns later than A-interleaved order, giving
        # the VectorE drain's in-flight PSUM reads a safe WAR margin (its
        # completion sem fires slightly before the last reads retire).
        if it in (0, 2, 4, 6):
            nc.tensor.wait_ge(s_wk[it // 2], 16)   # wts k-pair
        if k == 0:
            nc.tensor.wait_ge(s_p[t0], 16)
        if it >= 2:
            nc.tensor.wait_ge(s_act, it - 1)      # WAR: image0 banks free
        nc.tensor.matmul(PS[:, A0:A0 + 512], lhsT=lhsA, rhs=rhsA0,
                         start=True, stop=True)
        if k == 0:
            nc.tensor.wait_ge(s_p[t1], 16)
        nc.tensor.matmul(PS[:, A0 + 512:A0 + 1024], lhsT=lhsA, rhs=rhsA1,
                         start=True, stop=True).then_inc(s_pea)
        if it >= 2:
            nc.tensor.wait_ge(s_dve, it - 1)      # WAR: image1 banks free
        nc.tensor.matmul(PS[:, B0:B0 + 512], lhsT=lhsB, rhs=rhsB0,
                         start=True, stop=True)
        nc.tensor.matmul(PS[:, B0 + 512:B0 + 1024], lhsT=lhsB, rhs=rhsB1,
                         start=True, stop=True).then_inc(s_peb)

        # ScalarE drain: relu(x+b1), patch-sum into acc column
        if it == 0:
            nc.scalar.wait_ge(s_aux, 16)          # bias
        nc.scalar.wait_ge(s_pea, it + 1)
        nc.scalar.activation(
            PS[:, A0:A0 + 1024], PS[:, A0:A0 + 1024],
            mybir.ActivationFunctionType.Relu,
            bias=b1_ap, scale=1.0,
            accum_out=acc_t[:, 8 * k + g:8 * k + g + 1],
        ).then_inc(s_act)

        # VectorE drain: max(x+b1, 0), patch-sum into acc column
        if it == 0:
            nc.vector.wait_ge(s_aux, 16)          # bias
        nc.vector.wait_ge(s_peb, it + 1)
        nc.vector.scalar_tensor_tensor(
            out=PS[:, B0:B0 + 1024], in0=PS[:, B0:B0 + 1024],
            scalar=b1_ap, in1=zero_ap.to_broadcast([128, 1024]),
            op0=mybir.AluOpType.add, op1=mybir.AluOpType.max,
            accum_out=acc_t[:, 8 * k + 4 + g:8 * k + 4 + g + 1],
        ).then_inc(s_dve)

    # ---- tail
    # Fence on ScalarE: queue order guarantees every ACTIVATE's
    # accumulator-read aux op has retired (acc_t fully written) before
    # this fires; a bare then_inc on the last ACTIVATE could race its own
    # accumulator drain.
    nc.scalar.activation(
        warm_t, zero_ap[0:1, 0:1], mybir.ActivationFunctionType.Relu,
        bias=0.0, scale=1.0).then_inc(s_acc)

    # Vector-side spacers: the last STT's DVE_READ_ACCUMULATOR retires
    # just before this point but its SBUF write to acc_t can still be in
    # flight; two small ops give the writeback time to land before the
    # reduce reads acc_t (without them ~1 in 4 runs read stale columns).
    spacer_t = nc.alloc_sbuf_tensor("spacer_t", [1, 2], F32).ap()
    nc.vector.tensor_copy(out=spacer_t[0:1, 0:1], in_=zero_ap[0:1, 0:1])
    nc.vector.tensor_copy(out=spacer_t[0:1, 1:2], in_=zero_ap[0:1, 0:1])

    # s[:, 2k+img] = sum_g acc[:, 8k+4img+g]; bf16 out (4-term sums of
    # O(100) values -> ~0.4% rounding on s, well inside the 2e-2 gate)
    nc.vector.wait_ge(s_acc, 1)
    with nc.allow_low_precision("bf16 s_t feeds a bf16 matmul; 2e-2 gate"):
        nc.vector.tensor_reduce(
            out=s_t,
            in_=acc_t.rearrange("p (kh g) -> p kh g", g=4),
            axis=mybir.AxisListType.X,
            op=mybir.AluOpType.add,
        )
    # s_red fires from a follow-up op so the reduce's own s_t writeback has
    # landed before the PE's ldweights reads s_t.
    nc.vector.tensor_copy(
        out=spacer_t[0:1, 0:1], in_=zero_ap[0:1, 0:1]).then_inc(s_red)

    # out[img, :] = sum_k s[:, 2k+img]^T @ M2e_k + b2
    nc.tensor.wait_ge(s_m2, 16)
    nc.tensor.wait_ge(s_red, 1)
    for k in range(8):
        mm = nc.tensor.matmul(
            PS[0:2, 0:32],
            lhsT=s_t[:, 2 * k:2 * k + 2],
            rhs=m2_t[:, 32 * k:32 * k + 32],
            start=(k == 0), stop=(k == 7),
        )
    mm.then_inc(s_fin)

    nc.vector.wait_ge(s_fin, 1)
    nc.vector.tensor_tensor(
        out=out_sb, in0=PS[0:2, 0:32], in1=aux_t[0:2, AUXB2:AUXB2 + 32],
        op=mybir.AluOpType.add,
    ).then_inc(s_tt)

    nc.sync.wait_ge(s_tt, 1)
    nc.sync.dma_start(out=out_d[:, :], in_=out_sb).then_inc(s_out, 16)
    # no explicit end-of-kernel sem cleanup: the NEFF epilogue zeroes the
    # whole semaphore file, and the startup sem_clear covers first entry.

    nc.compile()
    return nc


_NC_CACHE = None
TRACE = False
_last_result = None
_last_profile_dir = None


def _get_nc():
    global _NC_CACHE
    if _NC_CACHE is None:
        _NC_CACHE = _build_nc()
    return _NC_CACHE


def kernel(x, w1, b1, w2, b2):
    global _last_result
    x = np.ascontiguousarray(np.asarray(x, dtype=np.float32))
    Wt, b1c, M2c, b2t = _build_weights(
        np.asarray(w1, np.float32), np.asarray(b1, np.float32),
        np.asarray(w2, np.float32), np.asarray(b2, np.float32))

    wts = np.empty((128, 1024), dtype=NP_BF16)
    wts[0:64] = Wt.astype(NP_BF16)
    wts[64:128] = wts[0:64]
    aux = np.empty((128, AUXTOT), dtype=np.float32)
    aux[:, AUXB1:AUXB1 + 8] = b1c
    aux[:, AUXB2:AUXB2 + 32] = b2t

    # patches: x [16,1,512,512] -> [b, pixel(r,c), patch(i,j)] = [16, 64, 4096]
    p_all = (x.reshape(16, 64, 8, 64, 8).transpose(0, 2, 4, 1, 3)
             .reshape(16, 64, 4096).astype(NP_BF16))

    in_maps = []
    for c in range(N_CORES):
        pc = np.empty((128, 4096), dtype=NP_BF16)
        pc[0:64] = p_all[2 * c]
        pc[64:128] = p_all[2 * c + 1]
        in_maps.append({"p": pc, "wts": wts, "aux": aux, "m2": M2c})

    nc = _get_nc()
    if TRACE:
        # Local profiling path: NTFF via direct ctypes calls into the axon
        # .so (this image's antenv lacks axon_hooks; the C ABI is stable).
        import ctypes
        import contextlib
        import tempfile
        from concourse import bass2jax

        @contextlib.contextmanager
        def _ntff_hook(output_dir, device_ids):
            import jax
            jax.devices()
            lib = ctypes.CDLL("/opt/axon/libaxon_pjrt.so")
            lib.axon_start_nrt_profile.argtypes = [
                ctypes.POINTER(ctypes.c_int64), ctypes.c_size_t]
            lib.axon_start_nrt_profile.restype = ctypes.c_int64
            lib.axon_stop_nrt_profile.argtypes = [ctypes.c_char_p]
            lib.axon_stop_nrt_profile.restype = ctypes.c_int64
            if device_ids:
                ids = (ctypes.c_int64 * len(device_ids))(*device_ids)
                rc = lib.axon_start_nrt_profile(ids, len(device_ids))
            else:
                rc = lib.axon_start_nrt_profile(None, 0)
            if rc != 0:
                raise RuntimeError(f"axon_start_nrt_profile rc={rc}")
            try:
                yield
            finally:
                n = lib.axon_stop_nrt_profile(str(output_dir).encode())
                print(f"profile: {n} file(s) written to {output_dir}")

        global _last_profile_dir
        tmpdir = tempfile.mkdtemp(prefix="dctcnn_prof_")
        with _ntff_hook(tmpdir, [0]):
            results = bass2jax.run_bass_via_pjrt(nc, in_maps, n_cores=N_CORES)
        _last_profile_dir = tmpdir
        out = np.concatenate([results[c]["out"] for c in range(N_CORES)], axis=0)
        return out.astype(np.float32)
    res = run_bass_kernel_spmd(nc, in_maps, list(range(N_CORES)))
    _last_result = res
    out = np.concatenate([res.results[c]["out"] for c in range(N_CORES)], axis=0)
    return out.astype(np.float32)
